# revision 1
# baseline (speedup 1.0000x reference)
"""AttLIF Trainium2 kernel: Linear(1024->2048) + temporal-attention gate + IF-neuron scan.

Self-contained: hardcodes shapes B=256, T=64, DIN=1024, DH=2048, 8 NeuronCores,
data-parallel over batch (32 batches/core).

Per core:
  x[bt, h] = dataE[bt, k] @ WE[k, h]     K extended with a ones/bias row
  avg[bt]  = dataE[bt, k] @ w_avg[k]     exact row-mean of x (w_avg = mean_h WE)
  mx[bt]   = max_h x[bt, h]              DVE reduce over 512-wide n-chunks
  score    = sigmoid(W2 @ (relu(W1@avg_b) + relu(W1@mx_b)))   tiny PE matmuls
  scan     : u = v + x*score; s = u>=0.6; v = u*(u<0.6)       DVE, T sequential

The GEMM runs in bf16 with a 3-term hi/lo split (x = d_hi@W_hi + d_hi@W_lo +
d_lo@W_hi, K-concatenated to one K=3073 GEMM) giving ~1.5e-5 relative error --
needed because the spike output is binary and flips near threshold; plain bf16
fails the tolerance, fp32 runs at 1/4 TensorE rate. Can switch to fp32/fp32r
via MODE.

Scan groups of BG batches: x lands in scan layout XS[p = b_l*HH + h_hi,
t*JW + j] via rearranging DMAs; spikes overwrite XS in place and stream out
per batch while the next group's GEMM runs on TensorE.
"""
import os
import sys
from contextlib import ExitStack

import numpy as np

sys.path.insert(0, "/opt/trn_rl_repo")

VTH = 0.6
B, T, DIN, DH = 256, 64, 1024, 2048
NCORES = 8
BS = B // NCORES   # 32
NM = BS * T // 128  # 16 m-tiles per core

MODE = os.environ.get("ATTLIF_MODE", "bf16x3")   # bf16x3 | fp32 | fp32r
BG = int(os.environ.get("ATTLIF_BG", "8"))       # batches per scan group


def _prep_weights(W, bias, W1, W2, mode):
    if mode == "bf16x3":
        import ml_dtypes
        bf = ml_dtypes.bfloat16
        Whi32 = W.astype(bf).astype(np.float32)
        Wlo = (W - Whi32).astype(bf).astype(np.float32)
        WE = np.concatenate([Whi32.T, Wlo.T, Whi32.T, bias[None, :]], axis=0)
        store = bf
    else:
        WE = np.concatenate([W.T, bias[None, :]], axis=0)
        store = np.float32
    KE = WE.shape[0]
    NK = (KE + 127) // 128
    KPAD = NK * 128
    WEp = np.zeros((KPAD, DH), np.float32)
    WEp[:KE] = WE
    wavg = WEp.mean(axis=1)
    wT = np.ascontiguousarray(
        WEp.reshape(NK, 128, DH // 512, 512).transpose(2, 0, 1, 3)).astype(store)
    wavg_arr = np.ascontiguousarray(wavg.reshape(NK, 128).T).astype(store)
    w1t = np.ascontiguousarray(W1.T).astype(np.float32)
    w2t = np.ascontiguousarray(W2.T).astype(np.float32)
    return dict(wT=wT, wavg=wavg_arr, w1t=w1t, w2t=w2t), NK, KPAD, store


def _prep_data_shard(shard, mode, NK, KPAD, store):
    rows = shard.reshape(BS * T, DIN).astype(np.float32)
    if mode == "bf16x3":
        import ml_dtypes
        bf = ml_dtypes.bfloat16
        dhi32 = rows.astype(bf).astype(np.float32)
        dlo = (rows - dhi32).astype(bf).astype(np.float32)
        dE = np.concatenate(
            [dhi32, dhi32, dlo, np.ones((BS * T, 1), np.float32)], axis=1)
    else:
        dE = np.concatenate([rows, np.ones((BS * T, 1), np.float32)], axis=1)
    dEp = np.zeros((BS * T, KPAD), np.float32)
    dEp[:, :dE.shape[1]] = dE
    return np.ascontiguousarray(
        dEp.reshape(NM, 128, NK, 128).transpose(0, 2, 3, 1)).astype(store)


def _build(nc, tile, mybir, op_dtype, NK, bg):
    f32 = mybir.dt.float32
    NG = BS // bg
    MG = bg // 2
    JW = (bg * DH) // 128
    HH = DH // JW
    NN = DH // 512
    aop = mybir.AluOpType

    dT = nc.dram_tensor("dT", [NM, NK, 128, 128], op_dtype, kind="ExternalInput").ap()
    wT = nc.dram_tensor("wT", [NN, NK, 128, 512], op_dtype, kind="ExternalInput").ap()
    wavg = nc.dram_tensor("wavg", [128, NK], op_dtype, kind="ExternalInput").ap()
    w1t = nc.dram_tensor("w1t", [T, 4], f32, kind="ExternalInput").ap()
    w2t = nc.dram_tensor("w2t", [4, T], f32, kind="ExternalInput").ap()
    out = nc.dram_tensor("out", [BS, T, DH], f32, kind="ExternalOutput").ap()

    with tile.TileContext(nc) as tc, ExitStack() as ctx:
        cpool = ctx.enter_context(tc.tile_pool(name="cpool", bufs=1))
        wpool = ctx.enter_context(tc.tile_pool(name="wpool", bufs=2))
        dpool = ctx.enter_context(tc.tile_pool(name="dpool", bufs=bg // 2 + 2))
        xmpool = ctx.enter_context(tc.tile_pool(name="xmpool", bufs=4))
        xspool = ctx.enter_context(tc.tile_pool(name="xspool", bufs=2))
        stpool = ctx.enter_context(tc.tile_pool(name="stpool", bufs=2))
        scpool = ctx.enter_context(tc.tile_pool(name="scpool", bufs=2))
        vpool = ctx.enter_context(tc.tile_pool(name="vpool", bufs=2))
        upool = ctx.enter_context(tc.tile_pool(name="upool", bufs=3))
        pgemm = ctx.enter_context(tc.tile_pool(name="pgemm", bufs=3, space="PSUM"))
        pavg = ctx.enter_context(tc.tile_pool(name="pavg", bufs=2, space="PSUM"))
        pmisc = ctx.enter_context(tc.tile_pool(name="pmisc", bufs=1, space="PSUM"))

        wavg_sb = cpool.tile([128, NK], op_dtype, name="wavg_sb")
        nc.sync.dma_start(wavg_sb[:], wavg[:])
        w1t_sb = cpool.tile([128, 4], f32, name="w1t_sb")
        nc.sync.dma_start(w1t_sb[0:T, :], w1t[:])
        nc.sync.dma_start(w1t_sb[T:128, :], w1t[:])
        w2t_sb = cpool.tile([4, T], f32, name="w2t_sb")
        nc.sync.dma_start(w2t_sb[:], w2t[:])

        for g in range(NG):
            XS = xspool.tile([128, T * JW], f32, name="XS", tag="XS")
            stats = stpool.tile([128, 2 * MG], f32, name="stats", tag="stats")
            rmx = stpool.tile([128, NN * MG], f32, name="rmx", tag="rmx")

            # load this group's stationary data tiles once (reused across all n)
            dts = []
            for ml in range(MG):
                dt = dpool.tile([128, NK * 128], op_dtype, name="dt", tag="dt")
                nc.sync.dma_start(dt[:], dT[g * MG + ml].rearrange("k kp j -> kp k j"))
                dts.append(dt)
            for n in range(NN):
                wc = wpool.tile([128, NK * 512], op_dtype, name="wc", tag="wc")
                nc.sync.dma_start(wc[:], wT[n].rearrange("k kp j -> kp k j"))
                for ml in range(MG):
                    dt = dts[ml]
                    ps = pgemm.tile([128, 512], f32, name="ps", tag="ps")
                    for k in range(NK):
                        nc.tensor.matmul(ps[:], dt[:, k * 128:(k + 1) * 128],
                                         wc[:, k * 512:(k + 1) * 512],
                                         start=(k == 0), stop=(k == NK - 1))
                    if n == 0:
                        pa = pavg.tile([128, 1], f32, name="pa", tag="pa")
                        for k in range(NK):
                            nc.tensor.matmul(pa[:], dt[:, k * 128:(k + 1) * 128],
                                             wavg_sb[:, k:k + 1],
                                             start=(k == 0), stop=(k == NK - 1))
                        nc.vector.tensor_copy(stats[:, ml:ml + 1], pa[:])
                    xm = xmpool.tile([128, 512], f32, name="xm", tag="xm")
                    nc.scalar.copy(xm[:], ps[:])
                    nc.vector.tensor_reduce(
                        rmx[:, ml * NN + n: ml * NN + n + 1], xm[:],
                        mybir.AxisListType.X, aop.max)
                    # scan layout: p = h_hi*BG + b_l, free = t*JW + j.
                    # Both bh halves in one DMA (dst partitions contiguous),
                    # issued on the otherwise-idle GpSimd queue.
                    nhh = 512 // JW
                    for h2 in range(nhh):
                        p0 = (n * nhh + h2) * bg + 2 * ml
                        nc.gpsimd.dma_start(
                            XS[p0:p0 + 2, :],
                            xm[:, h2 * JW:(h2 + 1) * JW])
            for ml in range(MG):
                nc.vector.tensor_reduce(
                    stats[:, MG + ml:MG + ml + 1],
                    rmx[:, ml * NN:(ml + 1) * NN],
                    mybir.AxisListType.X, aop.max)

            h1a = pmisc.tile([4, 2 * MG], f32, name="h1a", tag="pm1")
            nc.tensor.matmul(h1a[:], w1t_sb[0:T, :], stats[0:T, :],
                             start=True, stop=True)
            h1b = pmisc.tile([4, 2 * MG], f32, name="h1b", tag="pm2")
            nc.tensor.matmul(h1b[:], w1t_sb[T:128, :], stats[T:128, :],
                             start=True, stop=True)
            h1r = scpool.tile([4, 4 * MG], f32, name="h1r", tag="h1r")
            nc.scalar.activation(h1r[:, 0:2 * MG], h1a[:],
                                 mybir.ActivationFunctionType.Relu)
            nc.scalar.activation(h1r[:, 2 * MG:4 * MG], h1b[:],
                                 mybir.ActivationFunctionType.Relu)
            # Ht columns in natural batch order b_l = 2*ml + bh
            Ht = scpool.tile([4, 2 * MG], f32, name="Ht", tag="Ht")
            h4 = h1r[:].rearrange("r (b s m) -> r b s m", b=2, s=2)
            nc.vector.tensor_tensor(
                Ht[:].rearrange("r (m b) -> r b m", b=2), h4[:, :, 0], h4[:, :, 1],
                aop.add)
            # score directly in [b_l, t] layout: spT = Ht.T @ W2T
            spT = pmisc.tile([2 * MG, T], f32, name="spT", tag="pm1")
            nc.tensor.matmul(spT[:], Ht[:], w2t_sb[:], start=True, stop=True)
            scb = scpool.tile([2 * MG, T], f32, name="scb", tag="scb")
            nc.scalar.activation(scb[:], spT[:], mybir.ActivationFunctionType.Sigmoid)
            # replicate score rows to every h_hi block: ssc[hh*bg + b_l, t]
            ssc = scpool.tile([128, T], f32, name="ssc", tag="ssc")
            for hh in range(HH):
                nc.sync.dma_start(ssc[hh * bg:(hh + 1) * bg, :], scb[:])

            # scan: u_t = x_t*score + v (stored in place over x_t); v = u*(u<VTH)
            v = vpool.tile([128, JW], f32, name="v", tag="v")
            nc.vector.memset(v[:], 0.0)
            for t in range(T):
                xt = XS[:, t * JW:(t + 1) * JW]
                nc.vector.scalar_tensor_tensor(
                    xt, xt, ssc[:, t:t + 1], v[:], op0=aop.mult, op1=aop.add)
                nc.vector.scalar_tensor_tensor(
                    v[:], xt, VTH, xt, op0=aop.is_lt, op1=aop.mult)
            # bulk spike pass (all 128 partitions, one op), then stream out
            half = T * JW // 2
            for piece in range(2):
                nc.vector.tensor_scalar(
                    XS[:, piece * half:(piece + 1) * half],
                    XS[:, piece * half:(piece + 1) * half],
                    VTH, None, op0=aop.is_ge)
            for hh in range(HH):
                nc.sync.dma_start(
                    out[g * bg:(g + 1) * bg, :, hh * JW:(hh + 1) * JW],
                    XS[hh * bg:(hh + 1) * bg, :])


_CACHE = {}


def _get_compiled(mode, bg):
    key = (mode, bg)
    if key in _CACHE:
        return _CACHE[key]
    import concourse.tile as tile
    from concourse import bacc, mybir
    dtypes = {"fp32": mybir.dt.float32, "fp32r": mybir.dt.float32r,
              "bf16x3": mybir.dt.bfloat16}
    KE = 3 * DIN + 1 if mode == "bf16x3" else DIN + 1
    NK = (KE + 127) // 128
    nc = bacc.Bacc("TRN2", target_bir_lowering=False, debug=False, num_devices=1)
    _build(nc, tile, mybir, dtypes[mode], NK, bg)
    nc.compile()
    _CACHE[key] = nc
    return nc


def kernel(data, W, bias, W1, W2):
    from concourse.bass_utils import run_bass_kernel_spmd

    data = np.asarray(data, dtype=np.float32)
    W = np.asarray(W, dtype=np.float32)
    bias = np.asarray(bias, dtype=np.float32)
    W1 = np.asarray(W1, dtype=np.float32)
    W2 = np.asarray(W2, dtype=np.float32)

    wargs, NK, KPAD, store = _prep_weights(W, bias, W1, W2, MODE)
    in_maps = []
    for c in range(NCORES):
        shard = data[c * BS:(c + 1) * BS]
        dTc = _prep_data_shard(shard, MODE, NK, KPAD, store)
        in_maps.append({"dT": dTc, **wargs})

    nc = _get_compiled(MODE, BG)
    res = run_bass_kernel_spmd(nc, in_maps, core_ids=list(range(NCORES)))
    outs = [res.results[c]["out"] for c in range(NCORES)]
    return np.concatenate(outs, axis=0)


if __name__ == "__main__":
    rng = np.random.default_rng(0)
    d = rng.standard_normal((B, T, DIN)).astype(np.float32)
    w = (rng.standard_normal((DH, DIN)) / 32.0).astype(np.float32)
    b = np.zeros(DH, np.float32)
    w1 = (rng.standard_normal((4, T)) / 8.0).astype(np.float32)
    w2 = (rng.standard_normal((T, 4)) / 2.0).astype(np.float32)
    o = kernel(d, w, b, w1, w2)
    print(o.shape, o.dtype, o.mean())



# revision 4
# speedup vs baseline: 1.4676x; 1.4676x over previous
"""AttLIF Trainium2 kernel: Linear(1024->2048) + temporal-attention gate + IF scan.

Self-contained: B=256, T=64, DIN=1024, DH=2048, 8 NeuronCores, data-parallel
over batch (BS=32 per core). Per core, groups of bg=8 batches:

  GEMM    x[bt,h] = dE[bt,k] @ WE[k,h]   bf16 3-term hi/lo split (K=3073->3200)
          stationary = data k-tiles, moving = weight 512-chunks, PSUM f32
  avg     ACT engine computes it for free: PSUM->SBUF copy with accum_out
  mx      DVE reduce over 512-chunks, then over chunks
  score   sigmoid(W2 @ (relu(W1@avg) + relu(W1@mx)))  tiny PE matmuls
  scan    u = x*score + v; v = u*(u<VTH)  2 DVE stt ops per t
  spike   u >= VTH -> uint8, dumped in raw scan layout; host decodes

Layouts are host-prepped so every HBM load is contiguous per partition.
The (b,t)xh -> (h,b)x(t,h') rearrange runs as 64KB DMAs round-robined over
the sync/scalar/gpsimd queues, overlapped with the GEMM of the next chunk.
"""
import os
import sys
from contextlib import ExitStack

import numpy as np

sys.path.insert(0, "/opt/trn_rl_repo")

VTH = 0.6
B, T, DIN, DH = 256, 64, 1024, 2048
NCORES = 8
BS = B // NCORES     # 32
NM = BS * T // 128   # 16 m-tiles per core
KE = 3 * DIN + 1     # bf16x3 extended contraction (+1 bias row)
NK = (KE + 127) // 128   # 25
KPAD = NK * 128      # 3200
NN = DH // 512       # 4 n-chunks
BG = 8               # batches per scan group
MG = BG // 2         # 4 m-tiles per group
NG = BS // BG        # 4 groups
JW = BG * DH // 128  # 128 free elems per t in scan layout
HH = DH // JW        # 16 h_hi values


def _prep_weights(W, bias, W1, W2):
    import ml_dtypes
    bf = ml_dtypes.bfloat16
    Whi32 = W.astype(bf).astype(np.float32)
    Wlo = (W - Whi32).astype(bf).astype(np.float32)
    WE = np.concatenate([Whi32.T, Wlo.T, Whi32.T, bias[None, :]], axis=0)
    WEp = np.zeros((KPAD, DH), np.float32)
    WEp[:KE] = WE
    # per n-chunk: [kp, k*512+j] contiguous per partition
    wT = np.ascontiguousarray(
        WEp.reshape(NK, 128, NN, 512).transpose(2, 1, 0, 3)
        .reshape(NN, 128, NK * 512)).astype(bf)
    w1t = np.ascontiguousarray(W1.T).astype(np.float32)
    w2t = np.ascontiguousarray(W2.T).astype(np.float32)
    return dict(wT=wT, w1t=w1t, w2t=w2t)


def _prep_data_shard(shard):
    import ml_dtypes
    bf = ml_dtypes.bfloat16
    rows = shard.reshape(BS * T, DIN).astype(np.float32)
    dhi32 = rows.astype(bf).astype(np.float32)
    dlo = (rows - dhi32).astype(bf).astype(np.float32)
    dE = np.concatenate(
        [dhi32, dhi32, dlo, np.ones((BS * T, 1), np.float32)], axis=1)
    dEp = np.zeros((BS * T, KPAD), np.float32)
    dEp[:, :dE.shape[1]] = dE
    # per m-tile: [kp, k*128+m] contiguous per partition
    return np.ascontiguousarray(
        dEp.reshape(NM, 128, NK, 128).transpose(0, 3, 2, 1)
        .reshape(NM, 128, NK * 128)).astype(bf)


def _decode_out(dump):
    # dump u8 [NG, 128, T*JW]; partition p = hh*BG + bl, free = t*JW + j
    a = dump.reshape(NG, HH, BG, T, JW).transpose(0, 2, 3, 1, 4)
    return np.ascontiguousarray(a).reshape(BS, T, DH).astype(np.float32)


def _build(nc, tile, mybir):
    f32 = mybir.dt.float32
    bf16 = mybir.dt.bfloat16
    u8 = mybir.dt.uint8
    aop = mybir.AluOpType

    dT = nc.dram_tensor("dT", [NM, 128, NK * 128], bf16, kind="ExternalInput").ap()
    wT = nc.dram_tensor("wT", [NN, 128, NK * 512], bf16, kind="ExternalInput").ap()
    w1t = nc.dram_tensor("w1t", [T, 4], f32, kind="ExternalInput").ap()
    w2t = nc.dram_tensor("w2t", [4, T], f32, kind="ExternalInput").ap()
    outD = nc.dram_tensor("out", [NG, 128, T * JW], u8, kind="ExternalOutput").ap()

    with tile.TileContext(nc) as tc, ExitStack() as ctx:
        cpool = ctx.enter_context(tc.tile_pool(name="cpool", bufs=1))
        wpool = ctx.enter_context(tc.tile_pool(name="wpool", bufs=2))
        dpool = ctx.enter_context(tc.tile_pool(name="dpool", bufs=MG + 4))
        xmpool = ctx.enter_context(tc.tile_pool(name="xmpool", bufs=4))
        xspool = ctx.enter_context(tc.tile_pool(name="xspool", bufs=2))
        stpool = ctx.enter_context(tc.tile_pool(name="stpool", bufs=2))
        scpool = ctx.enter_context(tc.tile_pool(name="scpool", bufs=2))
        vpool = ctx.enter_context(tc.tile_pool(name="vpool", bufs=2))
        opool = ctx.enter_context(tc.tile_pool(name="opool", bufs=2))
        pgemm = ctx.enter_context(tc.tile_pool(name="pgemm", bufs=4, space="PSUM"))
        pmisc = ctx.enter_context(tc.tile_pool(name="pmisc", bufs=1, space="PSUM"))

        w1t_sb = cpool.tile([128, 4], f32, name="w1t_sb")
        nc.sync.dma_start(w1t_sb[0:T, :], w1t[:])
        nc.sync.dma_start(w1t_sb[T:128, :], w1t[:])
        w2t_sb = cpool.tile([4, T], f32, name="w2t_sb")
        nc.sync.dma_start(w2t_sb[:], w2t[:])

        rr_queues = [nc.sync, nc.scalar, nc.gpsimd]
        rr_i = 0

        for g in range(NG):
            XS = xspool.tile([128, T * JW], f32, name="XS", tag="XS")
            asum = stpool.tile([128, MG * NN], f32, name="asum", tag="asum")
            rmx = stpool.tile([128, MG * NN], f32, name="rmx", tag="rmx")
            stats = stpool.tile([128, 2 * MG], f32, name="stats", tag="stats")

            dts = []
            for ml in range(MG):
                dt = dpool.tile([128, NK * 128], bf16, name="dt", tag="dt")
                nc.sync.dma_start(dt[:], dT[g * MG + ml])
                dts.append(dt)
            for n in range(NN):
                wc = wpool.tile([128, NK * 512], bf16, name="wc", tag="wc")
                nc.gpsimd.dma_start(wc[:], wT[n])
                for ml in range(MG):
                    dt = dts[ml]
                    ps = pgemm.tile([128, 512], f32, name="ps", tag="ps")
                    for k in range(NK):
                        nc.tensor.matmul(ps[:], dt[:, k * 128:(k + 1) * 128],
                                         wc[:, k * 512:(k + 1) * 512],
                                         start=(k == 0), stop=(k == NK - 1))
                    xm = xmpool.tile([128, 512], f32, name="xm", tag="xm")
                    c = ml * NN + n
                    # PSUM->SBUF copy; ACT also emits the h-chunk sum (for avg)
                    nc.scalar.activation(
                        xm[:], ps[:], mybir.ActivationFunctionType.Copy,
                        accum_out=asum[:, c:c + 1])
                    nc.vector.tensor_reduce(
                        rmx[:, c:c + 1], xm[:], mybir.AxisListType.X, aop.max)
                    # scan layout: p = h_hi*BG + b_l, free = t*JW + j
                    for h2 in range(4):
                        p0 = (n * 4 + h2) * BG + 2 * ml
                        q = rr_queues[rr_i % 3]
                        rr_i += 1
                        q.dma_start(XS[p0:p0 + 2, :],
                                    xm[:, h2 * JW:(h2 + 1) * JW])
            # stats: avg = sum(asum)/DH, mx = max(rmx) per m-tile column
            nc.vector.tensor_reduce(
                stats[:, 0:MG], asum[:].rearrange("p (m n) -> p m n", n=NN),
                mybir.AxisListType.X, aop.add)
            nc.vector.tensor_scalar(
                stats[:, 0:MG], stats[:, 0:MG], 1.0 / DH, None, op0=aop.mult)
            nc.vector.tensor_reduce(
                stats[:, MG:2 * MG], rmx[:].rearrange("p (m n) -> p m n", n=NN),
                mybir.AxisListType.X, aop.max)

            h1a = pmisc.tile([4, 2 * MG], f32, name="h1a", tag="pm1")
            nc.tensor.matmul(h1a[:], w1t_sb[0:T, :], stats[0:T, :],
                             start=True, stop=True)
            h1b = pmisc.tile([4, 2 * MG], f32, name="h1b", tag="pm2")
            nc.tensor.matmul(h1b[:], w1t_sb[T:128, :], stats[T:128, :],
                             start=True, stop=True)
            h1r = scpool.tile([4, 4 * MG], f32, name="h1r", tag="h1r")
            nc.scalar.activation(h1r[:, 0:2 * MG], h1a[:],
                                 mybir.ActivationFunctionType.Relu)
            nc.scalar.activation(h1r[:, 2 * MG:4 * MG], h1b[:],
                                 mybir.ActivationFunctionType.Relu)
            # Ht columns in natural batch order b_l = 2*ml + b2
            Ht = scpool.tile([4, 2 * MG], f32, name="Ht", tag="Ht")
            h4 = h1r[:].rearrange("r (b s m) -> r b s m", b=2, s=2)
            nc.vector.tensor_tensor(
                Ht[:].rearrange("r (m b) -> r b m", b=2), h4[:, :, 0], h4[:, :, 1],
                aop.add)
            spT = pmisc.tile([2 * MG, T], f32, name="spT", tag="pm1")
            nc.tensor.matmul(spT[:], Ht[:], w2t_sb[:], start=True, stop=True)
            scb = scpool.tile([2 * MG, T], f32, name="scb", tag="scb")
            nc.scalar.activation(scb[:], spT[:],
                                 mybir.ActivationFunctionType.Sigmoid)
            ssc = scpool.tile([128, T], f32, name="ssc", tag="ssc")
            for hh in range(HH):
                nc.sync.dma_start(ssc[hh * BG:(hh + 1) * BG, :], scb[:])

            # scan: u_t = x_t*score + v (in place over x_t); v = u*(u<VTH)
            v = vpool.tile([128, JW], f32, name="v", tag="v")
            nc.vector.memset(v[:], 0.0)
            for t in range(T):
                xt = XS[:, t * JW:(t + 1) * JW]
                nc.vector.scalar_tensor_tensor(
                    xt, xt, ssc[:, t:t + 1], v[:], op0=aop.mult, op1=aop.add)
                nc.vector.scalar_tensor_tensor(
                    v[:], xt, VTH, xt, op0=aop.is_lt, op1=aop.mult)
            # spikes as u8 in raw scan layout; host decodes
            osb = opool.tile([128, T * JW], u8, name="osb", tag="osb")
            half = T * JW // 2
            for piece in range(2):
                nc.vector.tensor_scalar(
                    osb[:, piece * half:(piece + 1) * half],
                    XS[:, piece * half:(piece + 1) * half],
                    VTH, None, op0=aop.is_ge)
            nc.sync.dma_start(outD[g], osb[:])


_CACHE = {}


def _get_compiled():
    if "nc" in _CACHE:
        return _CACHE["nc"]
    import concourse.tile as tile
    from concourse import bacc, mybir
    nc = bacc.Bacc("TRN2", target_bir_lowering=False, debug=False, num_devices=1)
    _build(nc, tile, mybir)
    nc.compile()
    _CACHE["nc"] = nc
    return nc


def kernel(data, W, bias, W1, W2):
    from concourse.bass_utils import run_bass_kernel_spmd

    data = np.asarray(data, dtype=np.float32)
    W = np.asarray(W, dtype=np.float32)
    bias = np.asarray(bias, dtype=np.float32)
    W1 = np.asarray(W1, dtype=np.float32)
    W2 = np.asarray(W2, dtype=np.float32)

    wargs = _prep_weights(W, bias, W1, W2)
    in_maps = []
    for c in range(NCORES):
        shard = data[c * BS:(c + 1) * BS]
        in_maps.append({"dT": _prep_data_shard(shard), **wargs})

    nc = _get_compiled()
    res = run_bass_kernel_spmd(nc, in_maps, core_ids=list(range(NCORES)))
    outs = [_decode_out(res.results[c]["out"]) for c in range(NCORES)]
    return np.concatenate(outs, axis=0)


if __name__ == "__main__":
    rng = np.random.default_rng(0)
    d = rng.standard_normal((B, T, DIN)).astype(np.float32)
    w = (rng.standard_normal((DH, DIN)) / 32.0).astype(np.float32)
    b = np.zeros(DH, np.float32)
    w1 = (rng.standard_normal((4, T)) / 8.0).astype(np.float32)
    w2 = (rng.standard_normal((T, 4)) / 2.0).astype(np.float32)
    o = kernel(d, w, b, w1, w2)
    print(o.shape, o.dtype, o.mean())


# revision 7
# speedup vs baseline: 1.6506x; 1.1247x over previous
"""AttLIF Trainium2 kernel: Linear(1024->2048) + temporal-attention gate + IF scan.

Self-contained: B=256, T=64, DIN=1024, DH=2048, 8 NeuronCores, data-parallel
over batch (BS=32 per core). Per core, groups of bg=8 batches:

  GEMM    x[bt,h] = dE[bt,k] @ WE[k,h]   bf16 3-term hi/lo split (K=3073->3200)
          stationary = data k-tiles, moving = weight 512-chunks, PSUM f32
  avg     ACT engine computes it for free: PSUM->SBUF copy with accum_out
  mx      DVE reduce over 512-chunks, then over chunks
  score   sigmoid(W2 @ (relu(W1@avg) + relu(W1@mx)))  tiny PE matmuls
  scan    u = x*score + v; v = u*(u<VTH)  2 DVE stt ops per t
  spike   u >= VTH -> uint8, dumped in raw scan layout; host decodes

Layouts are host-prepped so every HBM load is contiguous per partition.
The (b,t)xh -> (h,b)x(t,h') rearrange runs as 64KB DMAs round-robined over
the sync/scalar/gpsimd queues, overlapped with the GEMM of the next chunk.
"""
import os
import sys
from contextlib import ExitStack

import numpy as np

sys.path.insert(0, "/opt/trn_rl_repo")

VTH = 0.6
B, T, DIN, DH = 256, 64, 1024, 2048
NCORES = 8
BS = B // NCORES     # 32
NM = BS * T // 128   # 16 m-tiles per core
KE = 3 * DIN + 1     # bf16x3 extended contraction (+1 bias row)
NK = (KE + 127) // 128   # 25
KPAD = NK * 128      # 3200
NN = DH // 512       # 4 n-chunks
BG = 8               # batches per scan group
MG = BG // 2         # 4 m-tiles per group
NG = BS // BG        # 4 groups
JW = BG * DH // 128  # 128 free elems per t in scan layout
HH = DH // JW        # 16 h_hi values


def _prep_weights(W, bias, W1, W2):
    import ml_dtypes
    bf = ml_dtypes.bfloat16
    Whi32 = W.astype(bf).astype(np.float32)
    Wlo = (W - Whi32).astype(bf).astype(np.float32)
    WE = np.concatenate([Whi32.T, Wlo.T, Whi32.T, bias[None, :]], axis=0)
    WEp = np.zeros((KPAD, DH), np.float32)
    WEp[:KE] = WE
    # per n-chunk: [kp, k*512+j] contiguous per partition
    wT = np.ascontiguousarray(
        WEp.reshape(NK, 128, NN, 512).transpose(2, 1, 0, 3)
        .reshape(NN, 128, NK * 512)).astype(bf)
    w1t = np.ascontiguousarray(W1.T).astype(np.float32)
    w2t = np.ascontiguousarray(W2.T).astype(np.float32)
    return dict(wT=wT, w1t=w1t, w2t=w2t)


def _prep_data_shard(shard):
    import ml_dtypes
    bf = ml_dtypes.bfloat16
    rows = shard.reshape(BS * T, DIN).astype(np.float32)
    dhi32 = rows.astype(bf).astype(np.float32)
    dlo = (rows - dhi32).astype(bf).astype(np.float32)
    dE = np.concatenate(
        [dhi32, dhi32, dlo, np.ones((BS * T, 1), np.float32)], axis=1)
    dEp = np.zeros((BS * T, KPAD), np.float32)
    dEp[:, :dE.shape[1]] = dE
    # per m-tile: [kp, k*128+m] contiguous per partition
    return np.ascontiguousarray(
        dEp.reshape(NM, 128, NK, 128).transpose(0, 3, 2, 1)
        .reshape(NM, 128, NK * 128)).astype(bf)


def _decode_out(dump):
    # dump u8 [NG, 128, T*JW]; partition p = hh*BG + bl, free = t*JW + j
    a = dump.reshape(NG, HH, BG, T, JW).transpose(0, 2, 3, 1, 4)
    return np.ascontiguousarray(a).reshape(BS, T, DH).astype(np.float32)


def _build(nc, tile, mybir):
    f32 = mybir.dt.float32
    bf16 = mybir.dt.bfloat16
    u8 = mybir.dt.uint8
    aop = mybir.AluOpType

    dT = nc.dram_tensor("dT", [NM, 128, NK * 128], bf16, kind="ExternalInput").ap()
    wT = nc.dram_tensor("wT", [NN, 128, NK * 512], bf16, kind="ExternalInput").ap()
    w1t = nc.dram_tensor("w1t", [T, 4], f32, kind="ExternalInput").ap()
    w2t = nc.dram_tensor("w2t", [4, T], f32, kind="ExternalInput").ap()
    outD = nc.dram_tensor("out", [NG, 128, T * JW], u8, kind="ExternalOutput").ap()

    with tile.TileContext(nc) as tc, ExitStack() as ctx:
        cpool = ctx.enter_context(tc.tile_pool(name="cpool", bufs=1))
        wpool = ctx.enter_context(tc.tile_pool(name="wpool", bufs=2))
        dpool = ctx.enter_context(tc.tile_pool(name="dpool", bufs=MG + 4))
        xmpool = ctx.enter_context(tc.tile_pool(name="xmpool", bufs=4))
        xspool = ctx.enter_context(tc.tile_pool(name="xspool", bufs=2))
        stpool = ctx.enter_context(tc.tile_pool(name="stpool", bufs=2))
        scpool = ctx.enter_context(tc.tile_pool(name="scpool", bufs=2))
        vpool = ctx.enter_context(tc.tile_pool(name="vpool", bufs=2))
        opool = ctx.enter_context(tc.tile_pool(name="opool", bufs=2))
        pgemm = ctx.enter_context(tc.tile_pool(name="pgemm", bufs=4, space="PSUM"))
        pmisc = ctx.enter_context(tc.tile_pool(name="pmisc", bufs=1, space="PSUM"))

        w1t_sb = cpool.tile([128, 4], f32, name="w1t_sb")
        nc.sync.dma_start(w1t_sb[0:T, :], w1t[:])
        nc.sync.dma_start(w1t_sb[T:128, :], w1t[:])
        w2t_sb = cpool.tile([4, T], f32, name="w2t_sb")
        nc.sync.dma_start(w2t_sb[:], w2t[:])



        for g in range(NG):
            XS = xspool.tile([128, T * JW], f32, name="XS", tag="XS")
            asum = stpool.tile([128, MG * NN], f32, name="asum", tag="asum")
            rmx = stpool.tile([128, MG * NN], f32, name="rmx", tag="rmx")
            stats = stpool.tile([128, 2 * MG], f32, name="stats", tag="stats")

            dts = []
            for ml in range(MG):
                dt = dpool.tile([128, NK * 128], bf16, name="dt", tag="dt")
                nc.sync.dma_start(dt[:], dT[g * MG + ml])
                dts.append(dt)
            for n in range(NN):
                wc = wpool.tile([128, NK * 512], bf16, name="wc", tag="wc")
                nc.gpsimd.dma_start(wc[:], wT[n])
                for ml in range(MG):
                    dt = dts[ml]
                    ps = pgemm.tile([128, 512], f32, name="ps", tag="ps")
                    for k in range(NK):
                        nc.tensor.matmul(ps[:], dt[:, k * 128:(k + 1) * 128],
                                         wc[:, k * 512:(k + 1) * 512],
                                         start=(k == 0), stop=(k == NK - 1))
                    xm = xmpool.tile([128, 512], f32, name="xm", tag="xm")
                    c = ml * NN + n
                    # PSUM->SBUF copy; ACT also emits the h-chunk sum (for avg)
                    nc.scalar.activation(
                        xm[:], ps[:], mybir.ActivationFunctionType.Copy,
                        accum_out=asum[:, c:c + 1])
                    nc.vector.tensor_reduce(
                        rmx[:, c:c + 1], xm[:], mybir.AxisListType.X, aop.max)
                    # scan layout: p = h_hi*BG + b_l, free = t*JW + j
                    # SWDGE trigger is ~1us; transfers run async on SDMA rings
                    for h2 in range(4):
                        p0 = (n * 4 + h2) * BG + 2 * ml
                        nc.gpsimd.dma_start(XS[p0:p0 + 2, :],
                                            xm[:, h2 * JW:(h2 + 1) * JW])
            # stats: avg = sum(asum)/DH, mx = max(rmx) per m-tile column
            nc.vector.tensor_reduce(
                stats[:, 0:MG], asum[:].rearrange("p (m n) -> p m n", n=NN),
                mybir.AxisListType.X, aop.add)
            nc.vector.tensor_scalar(
                stats[:, 0:MG], stats[:, 0:MG], 1.0 / DH, None, op0=aop.mult)
            nc.vector.tensor_reduce(
                stats[:, MG:2 * MG], rmx[:].rearrange("p (m n) -> p m n", n=NN),
                mybir.AxisListType.X, aop.max)

            h1a = pmisc.tile([4, 2 * MG], f32, name="h1a", tag="pm1")
            nc.tensor.matmul(h1a[:], w1t_sb[0:T, :], stats[0:T, :],
                             start=True, stop=True)
            h1b = pmisc.tile([4, 2 * MG], f32, name="h1b", tag="pm2")
            nc.tensor.matmul(h1b[:], w1t_sb[T:128, :], stats[T:128, :],
                             start=True, stop=True)
            h1r = scpool.tile([4, 4 * MG], f32, name="h1r", tag="h1r")
            nc.scalar.activation(h1r[:, 0:2 * MG], h1a[:],
                                 mybir.ActivationFunctionType.Relu)
            nc.scalar.activation(h1r[:, 2 * MG:4 * MG], h1b[:],
                                 mybir.ActivationFunctionType.Relu)
            # Ht columns in natural batch order b_l = 2*ml + b2
            Ht = scpool.tile([4, 2 * MG], f32, name="Ht", tag="Ht")
            h4 = h1r[:].rearrange("r (b s m) -> r b s m", b=2, s=2)
            nc.vector.tensor_tensor(
                Ht[:].rearrange("r (m b) -> r b m", b=2), h4[:, :, 0], h4[:, :, 1],
                aop.add)
            spT = pmisc.tile([2 * MG, T], f32, name="spT", tag="pm1")
            nc.tensor.matmul(spT[:], Ht[:], w2t_sb[:], start=True, stop=True)
            scb = scpool.tile([2 * MG, T], f32, name="scb", tag="scb")
            nc.scalar.activation(scb[:], spT[:],
                                 mybir.ActivationFunctionType.Sigmoid)
            ssc = scpool.tile([128, T], f32, name="ssc", tag="ssc")
            nc.scalar.dma_start(ssc[0:BG, :], scb[:])
            for m in (1, 2, 4, 8):   # log-doubling partition replicate
                nc.scalar.dma_start(ssc[m * BG:2 * m * BG, :], ssc[0:m * BG, :])

            # scan: u_t = x_t*score + v (in place over x_t); v = u*(u<VTH)
            v = vpool.tile([128, JW], f32, name="v", tag="v")
            nc.vector.memset(v[:], 0.0)
            for t in range(T):
                xt = XS[:, t * JW:(t + 1) * JW]
                nc.vector.scalar_tensor_tensor(
                    xt, xt, ssc[:, t:t + 1], v[:], op0=aop.mult, op1=aop.add)
                nc.vector.scalar_tensor_tensor(
                    v[:], xt, VTH, xt, op0=aop.is_lt, op1=aop.mult)
            # spikes as u8 in raw scan layout; host decodes
            osb = opool.tile([128, T * JW], u8, name="osb", tag="osb")
            half = T * JW // 2
            for piece in range(2):
                nc.vector.tensor_scalar(
                    osb[:, piece * half:(piece + 1) * half],
                    XS[:, piece * half:(piece + 1) * half],
                    VTH, None, op0=aop.is_ge)
            nc.sync.dma_start(outD[g], osb[:])


_CACHE = {}


def _get_compiled():
    if "nc" in _CACHE:
        return _CACHE["nc"]
    import concourse.tile as tile
    from concourse import bacc, mybir
    nc = bacc.Bacc("TRN2", target_bir_lowering=False, debug=False, num_devices=1)
    _build(nc, tile, mybir)
    nc.compile()
    _CACHE["nc"] = nc
    return nc


def kernel(data, W, bias, W1, W2):
    from concourse.bass_utils import run_bass_kernel_spmd

    data = np.asarray(data, dtype=np.float32)
    W = np.asarray(W, dtype=np.float32)
    bias = np.asarray(bias, dtype=np.float32)
    W1 = np.asarray(W1, dtype=np.float32)
    W2 = np.asarray(W2, dtype=np.float32)

    wargs = _prep_weights(W, bias, W1, W2)
    in_maps = []
    for c in range(NCORES):
        shard = data[c * BS:(c + 1) * BS]
        in_maps.append({"dT": _prep_data_shard(shard), **wargs})

    nc = _get_compiled()
    res = run_bass_kernel_spmd(nc, in_maps, core_ids=list(range(NCORES)))
    outs = [_decode_out(res.results[c]["out"]) for c in range(NCORES)]
    return np.concatenate(outs, axis=0)


if __name__ == "__main__":
    rng = np.random.default_rng(0)
    d = rng.standard_normal((B, T, DIN)).astype(np.float32)
    w = (rng.standard_normal((DH, DIN)) / 32.0).astype(np.float32)
    b = np.zeros(DH, np.float32)
    w1 = (rng.standard_normal((4, T)) / 8.0).astype(np.float32)
    w2 = (rng.standard_normal((T, 4)) / 2.0).astype(np.float32)
    o = kernel(d, w, b, w1, w2)
    print(o.shape, o.dtype, o.mean())
